# revision 20
# baseline (speedup 1.0000x reference)
"""Trainium2 Bass kernel: 12-head attention block (qkv proj -> softmax attn -> fc).

Reference semantics (B=32, S=577, D=768, H=12, Dh=64):
    qkv = x @ w_qkv + b_qkv
    q, k, v = split(qkv); attn = softmax(q k^T / 8) v
    out = attn @ w_fc + b_fc

Sharding: data-parallel over batch across 8 NeuronCores (4 images per core),
weights replicated, no collectives. Compute in bf16 with fp32 PSUM accumulation.

v3 layout strategy per core (all matmuls contract over the partition dim):
  - x is transposed by the DMA XBAR (dma_start_transpose, bf16) into
    xT_all [128, (si dk sl)] -- zero PE/DVE cost. Row block si=4 loads rows
    449:577 so every block is a full 128 rows. x0 arrives f32 on the sync
    HWDGE queue and is cast bf16 on the (startup-idle) ScalarE; x1..3 are
    cast in flight by the gpsimd SWDGE queue.
  - w_qkv streams in k-blocks [128, 2304] split into two column halves on
    two parallel queues: cols 0:1152 f32 on the scalar HWDGE queue (+
    ScalarE cast), cols 1152:2304 (incl. v) casting-DMA on SWDGE. Full-row
    descriptors keep both streams near peak DMA bandwidth.
  - qkT [1536, 577] = w_qkv[:, :1536]^T . xT; evac PSUM->bf16 with
    per-partition bias on DVE (tensor_scalar_add).
  - v [577, 768] natural + per-head ones column so attention row-sums fall
    out of the attn@v matmul for free.
  - scoresT[sk, sq] = kT_h^T . qT_h; heads paired even/odd on disjoint PE
    row groups (concurrent K=64 matmuls). exp on ScalarE (scale folded in).
  - attn_outT[65, sq] = (v_h|1)^T . expT; row 64 = softmax denominators.
  - normalize: reciprocal_approx_fast (DVE) + partition_broadcast + multiply
    on gpsimd (keeps DVE free for PSUM evacuation).
  - fc: out[s, :] = attn_T_k^T . w_fc_k, + b_fc broadcast.

Scheduling: attention alternates PE-light/ScalarE-heavy (scores+exp) with
PE-heavy (attn@v) phases; independent PE work (late qkT tiles, prev-batch fc,
next-batch qkT/v) is woven as filler between scores sk-groups, PACED EVENLY
across the 30 weave sites per batch so the PE stream stays dense end-to-end
(HAM clock-gate stays at full rate). PSUM: two 2-slot pools, 8 banks total.
"""

import os
import sys

import numpy as np

for _p in ("/opt/trn_rl_repo", "/root/.axon_site/_ro/trn_rl_repo"):
    if os.path.isdir(_p) and _p not in sys.path:
        sys.path.insert(0, _p)

import concourse.bass as bass  # noqa: E402
import concourse.tile as tile  # noqa: E402
from concourse import bacc, mybir  # noqa: E402
from concourse.bass_utils import run_bass_kernel_spmd  # noqa: E402
from concourse.masks import make_identity  # noqa: E402

F32 = mybir.dt.float32
BF16 = mybir.dt.bfloat16

B, S, D = 32, 577, 768
H, DH = 12, 64
NCORES = 8
NB = B // NCORES  # 4 batch images per core
SCALE = DH**-0.5  # 0.125
NKT = D // 128  # 6 contraction tiles of 128
S_TILES = [(0, 128), (128, 128), (256, 128), (384, 128), (512, 65)]
# x row blocks for DMA/transpose: all full 128 rows (block 4 overlaps block 3)
X_TILES = [0, 128, 256, 384, 449]
SL4 = 512 - 449  # si=4: column offset of s=512 within its transposed block
CH_S = [(0, 512), (512, 65)]  # 577 split at PSUM-bank boundary
CH_D = [(0, 512), (512, 256)]  # 768 split at PSUM-bank boundary
WSPLIT = 1152  # w_qkv column split between the two startup DMA streams
EXP = mybir.ActivationFunctionType.Exp
COPY = mybir.ActivationFunctionType.Copy


def build_nc():
    nc = bacc.Bacc(None)
    x_ext = nc.declare_dram_parameter("x", [NB, S, D], F32, isOutput=False)
    wqkv_ext = nc.declare_dram_parameter("w_qkv", [D, 3 * D], F32, isOutput=False)
    bqkv_ext = nc.declare_dram_parameter("b_qkv", [3 * D], F32, isOutput=False)
    wfc_ext = nc.declare_dram_parameter("w_fc", [D, D], F32, isOutput=False)
    bfc_ext = nc.declare_dram_parameter("b_fc", [D], F32, isOutput=False)
    out_ext = nc.declare_dram_parameter("out", [NB, S, D], F32, isOutput=True)

    with tile.TileContext(nc) as tc:
        with (
            tc.tile_pool(name="const", bufs=1) as cpool,
            tc.tile_pool(name="x", bufs=2) as x_pool,
            tc.tile_pool(name="wstg", bufs=2) as wstg_pool,
            tc.tile_pool(name="xT", bufs=2) as xT_pool,
            tc.tile_pool(name="qkT", bufs=2) as qkT_pool,
            tc.tile_pool(name="v", bufs=2) as v_pool,
            tc.tile_pool(name="expT", bufs=5) as expT_pool,
            tc.tile_pool(name="attnT", bufs=2) as attnT_pool,
            tc.tile_pool(name="small", bufs=3) as small_pool,
            tc.tile_pool(name="osb", bufs=3) as osb_pool,
            tc.tile_pool(name="psS", bufs=2, space="PSUM") as psS,
            tc.tile_pool(name="psW", bufs=2, space="PSUM") as psW,
        ):
            x_t = {}
            # w_qkv bf16 k-blocks [128, 2304] (q | k | v columns)
            w_qkv_k = [
                cpool.tile([128, 3 * D], BF16, name=f"wqkv{k}") for k in range(NKT)
            ]
            wfc_k = [cpool.tile([128, D], BF16, name=f"wfc{k}") for k in range(NKT)]

            # ---- startup streams ----
            # constants FIRST: the identity build must head the gpsimd queue
            # (behind the SWDGE descriptor generation it would stall the
            # b_qk matmul at the head of the in-order PE queue for ~25us)
            identity = cpool.tile([12, 12], F32)
            make_identity(nc, identity[:])
            ones = cpool.tile([1, 128], F32)
            nc.vector.memset(ones[:], 1.0)
            # tiny bias DMAs first on sync so they aren't stuck behind staging
            b12 = cpool.tile([12, 128], F32)
            nc.sync.dma_start(
                b12[:], bqkv_ext[0 : 2 * D].rearrange("(m p) -> m p", m=12)
            )
            brow_v = cpool.tile([1, D], F32)
            nc.sync.dma_start(brow_v[:], bqkv_ext[None, 2 * D : 3 * D])
            brow_fc = cpool.tile([1, D], F32)
            nc.sync.dma_start(brow_fc[:], bfc_ext[None, :])

            def emit_x_dma(b):  # SWDGE casting x load
                x_t[b] = x_pool.tile([128, 5 * D], BF16, tag="x", name="x_all")
                for si, xs0 in enumerate(X_TILES):
                    nc.gpsimd.dma_start(
                        x_t[b][:, si * D : (si + 1) * D],
                        x_ext[b, xs0 : xs0 + 128, :],
                    )

            # SWDGE: x0 first (gates ALL early PE work), then v columns
            # (attn@v p0), then the m9..11 k-head columns (scores pair 3+,
            # not read until ~45us)
            emit_x_dma(0)
            for k in range(NKT):
                nc.gpsimd.dma_start(
                    w_qkv_k[k][:, 2 * D : 3 * D],
                    wqkv_ext[k * 128 : (k + 1) * 128, 2 * D : 3 * D],
                )
            for k in range(NKT):
                nc.gpsimd.dma_start(
                    w_qkv_k[k][:, WSPLIT : 2 * D],
                    wqkv_ext[k * 128 : (k + 1) * 128, WSPLIT : 2 * D],
                )
            # w_qkv cols 0:1152 f32 staged via BOTH HWDGE queues in parallel,
            # cast on DVE (ScalarE carries ONLY exp: casts there get
            # priority-inverted behind exps that transitively need them)
            wstg = {}
            for k in range(NKT):
                wstg[k] = wstg_pool.tile([128, WSPLIT], F32, tag="wstg", name="wstg")
                eng = nc.sync if k % 2 == 0 else nc.scalar
                eng.dma_start(
                    wstg[k][:], wqkv_ext[k * 128 : (k + 1) * 128, 0:WSPLIT]
                )
            for k in range(NKT):
                nc.vector.tensor_copy(w_qkv_k[k][:, 0:WSPLIT], wstg[k][:])

            # b_qk: PE-transpose the [12,128] bias block to [128,12]
            b_qk = cpool.tile([128, H], F32)
            pbk = psW.tile([128, H], F32, tag="psW", name="pbk")
            nc.tensor.matmul(
                pbk[:, :], lhsT=b12[:, :], rhs=identity[:, :], start=True, stop=True
            )
            nc.vector.tensor_copy(b_qk[:], pbk[:])

            # broadcast biases to all 128 partitions via K=1 matmul
            b_v_bc = cpool.tile([128, D], F32)
            b_fc_bc = cpool.tile([128, D], F32)
            for row, bc in ((brow_v, b_v_bc), (brow_fc, b_fc_bc)):
                pb = psW.tile([128, D], F32, tag="psW", name="pb")
                for c0, cl in CH_D:
                    nc.tensor.matmul(
                        pb[:, c0 : c0 + cl],
                        lhsT=ones[0:1, 0:128],
                        rhs=row[0:1, c0 : c0 + cl],
                        start=True,
                        stop=True,
                    )
                nc.vector.tensor_copy(bc[:], pb[:])

            # ---- xT via DMA XBAR transpose (sync queue) ----
            # xT_all[b][p, si*768 + dk*128 + sl] = x[X_TILES[si]+sl, dk*128+p]
            xT_t = {}

            def emit_xT(b, eng=None):
                # b=0 issues from the scalar queue (nothing queued behind it
                # there); later batches from sync where x_t is already
                # resident by emission time so no head-of-line blocking
                eng = eng or nc.sync
                xT_t[b] = xT_pool.tile([128, 5 * NKT * 128], BF16, tag="xT", name="xT")
                for si in range(5):
                    eng.dma_start_transpose(
                        xT_t[b][:, si * D : (si + 1) * D].rearrange(
                            "p (dk sl) -> p dk sl", dk=NKT
                        ),
                        x_t[b][:, si * D : (si + 1) * D],
                    )

            def xT4(b):
                return xT_t[b][:].rearrange("p (si dk sl) -> p si dk sl", si=5, dk=NKT)

            # rhs APs for qkT: s-chunk (0,512) spans si 0..3; (512,65) is si4
            def xT_schunk(b, k, c0):
                if c0 == 0:
                    return xT4(b)[:, 0:4, k, :]
                return xT4(b)[:, 4, k, SL4 : SL4 + 65]

            # ---- per-batch emission helpers ----
            qkT_t = {}

            def emit_qkT_mtile(b, m):
                if b not in qkT_t:
                    qkT_t[b] = {}
                qkT_t[b][m] = qkT_pool.tile(
                    [128, S], BF16, tag=f"qkT{m}", name=f"qkT{m}"
                )
                pqk = psW.tile([128, S], F32, tag="psW", name="pqk")
                for k in range(NKT):
                    for c0, cl in CH_S:
                        nc.tensor.matmul(
                            pqk[:, c0 : c0 + cl],
                            lhsT=w_qkv_k[k][:, m * 128 : (m + 1) * 128],
                            rhs=xT_schunk(b, k, c0),
                            start=(k == 0),
                            stop=(k == NKT - 1),
                        )
                nc.vector.tensor_scalar_add(qkT_t[b][m][:], pqk[:], b_qk[:, m : m + 1])

            def emit_v(b):
                # v natural [577, 768] + per-head ones column (65 floats per head)
                v_all = v_pool.tile(
                    [128, 5 * H * (DH + 1)], BF16, tag="v", name="v_all"
                )
                v4 = v_all[:].rearrange("p (s h e) -> p s h e", s=5, h=H)
                nc.vector.memset(v4[:, :, :, DH : DH + 1], 1.0)
                for si, (s0, psl) in enumerate(S_TILES):
                    pv = psW.tile([128, D], F32, tag="psW", name="pv")
                    sl0 = SL4 if si == 4 else 0
                    for k in range(NKT):
                        for c0, cl in CH_D:
                            nc.tensor.matmul(
                                pv[0:psl, c0 : c0 + cl],
                                lhsT=xT4(b)[:, si, k, sl0 : sl0 + psl],
                                rhs=w_qkv_k[k][:, 2 * D + c0 : 2 * D + c0 + cl],
                                start=(k == 0),
                                stop=(k == NKT - 1),
                            )
                    nc.vector.tensor_add(
                        v4[0:psl, si, :, 0:DH],
                        pv[0:psl, :].rearrange("p (h e) -> p h e", h=H),
                        b_v_bc[0:psl, :].rearrange("p (h e) -> p h e", h=H),
                    )
                return v_all

            def emit_scores(p, qkT_all, expT, try_fill=None):
                heads = (2 * p, 2 * p + 1)
                for h in heads:
                    expT[h] = expT_pool.tile(
                        [128, 5 * S], BF16, tag="expT", name=f"expT{h % 2}"
                    )
                for si, (s0, psl) in enumerate(S_TILES):
                    psc = {}
                    for h in heads:
                        psc[h] = psS.tile([128, S], F32, tag="psS", name=f"psc{h % 2}")
                    for c0, cl in CH_S:
                        for h in heads:
                            hoff = (h % 2) * 64
                            qm, km = h // 2, NKT + h // 2
                            nc.tensor.matmul(
                                psc[h][0:psl, c0 : c0 + cl],
                                lhsT=qkT_all[km][hoff : hoff + 64, s0 : s0 + psl],
                                rhs=qkT_all[qm][hoff : hoff + 64, c0 : c0 + cl],
                                start=True,
                                stop=True,
                            )
                    for h in heads:
                        nc.scalar.activation(
                            expT[h][0:psl, si * S : (si + 1) * S],
                            psc[h][0:psl, :],
                            EXP,
                            scale=float(SCALE),
                        )
                    if try_fill is not None:
                        try_fill()

            def emit_attnv(p, v_all, attnT_all, expT):
                heads = (2 * p, 2 * p + 1)
                rinv = {}
                for h in heads:
                    hoff = (h % 2) * 64
                    # attn_outT [65, 577]: rows 0:64 = out^T unnorm, row 64 = sums
                    po = psW.tile([65, S], F32, tag="psW", name="po")
                    for si, (s0, psl) in enumerate(S_TILES):
                        for c0, cl in CH_S:
                            nc.tensor.matmul(
                                po[:, c0 : c0 + cl],
                                lhsT=v_all[
                                    0:psl,
                                    si * H * (DH + 1)
                                    + h * (DH + 1) : si * H * (DH + 1)
                                    + (h + 1) * (DH + 1),
                                ],
                                rhs=expT[h][0:psl, si * S + c0 : si * S + c0 + cl],
                                start=(si == 0),
                                stop=(si == 4),
                            )
                    # drain po fast: unnormalized copy + staged fast reciprocal
                    nc.vector.tensor_copy(
                        attnT_all[hoff : hoff + 64, (h // 2) * S : (h // 2 + 1) * S],
                        po[0:64, :],
                    )
                    rs = small_pool.tile([1, S], F32, tag="rs", name=f"rs{h % 2}")
                    nc.vector.tensor_copy(rs[:], po[64:65, :])
                    rinv[h] = small_pool.tile(
                        [1, S], F32, tag="rinv", name=f"rinv{h % 2}"
                    )
                    nc.vector.reciprocal_approx_fast(rinv[h][:], rs[:])
                    del expT[h]
                for h in heads:
                    hoff = (h % 2) * 64
                    rbc = small_pool.tile([128, S], F32, tag="rbc")
                    nc.gpsimd.partition_broadcast(rbc[:, :], rinv[h][0:1, :])
                    nc.vector.tensor_mul(
                        attnT_all[hoff : hoff + 64, (h // 2) * S : (h // 2 + 1) * S],
                        attnT_all[hoff : hoff + 64, (h // 2) * S : (h // 2 + 1) * S],
                        rbc[hoff : hoff + 64, :],
                    )

            def emit_fc_si(b, attnT_all, si):
                s0, psl = S_TILES[si]
                pf = psW.tile([128, D], F32, tag="psW", name="pf")
                for k in range(NKT):
                    for c0, cl in CH_D:
                        nc.tensor.matmul(
                            pf[0:psl, c0 : c0 + cl],
                            lhsT=attnT_all[:, k * S + s0 : k * S + s0 + psl],
                            rhs=wfc_k[k][:, c0 : c0 + cl],
                            start=(k == 0),
                            stop=(k == NKT - 1),
                        )
                osb = osb_pool.tile([128, D], F32, tag="osb")
                nc.vector.tensor_add(osb[0:psl, :], pf[0:psl, :], b_fc_bc[0:psl, :])
                nc.sync.dma_start(out_ext[b, s0 : s0 + psl, :], osb[0:psl, :])

            # ---- schedule ----
            emit_xT(0, nc.scalar)
            # scores pair 0/1 tiles up front; rest woven as fillers
            for m in (0, 6, 1, 7):
                emit_qkT_mtile(0, m)
            # SWDGE continues: x1, then wfc (needed ~100us in), then x2/x3
            # are emitted inside the batch loop
            emit_x_dma(1)
            emit_xT(1)
            for k in range(NKT):
                nc.gpsimd.dma_start(wfc_k[k][:], wfc_ext[k * 128 : (k + 1) * 128, :])
            v_t = {}
            attnT_t = {}
            v_t[0] = emit_v(0)

            N_SITES = (H // 2) * 5  # try_fill call sites per batch

            for b in range(NB):
                fillers = []
                # this batch's remaining qkT tiles, 2 per step, one step ahead
                # of the scores pair that reads them (pair p needs m=p, 6+p)
                for p in range(2, NKT):
                    fillers.append(
                        (lambda bb, mm: lambda: emit_qkT_mtile(bb, mm))(b, p)
                    )
                    fillers.append(
                        (lambda bb, mm: lambda: emit_qkT_mtile(bb, mm))(b, NKT + p)
                    )
                if b >= 1:
                    for si in range(5):
                        fillers.append(
                            (lambda bb, ss: lambda: emit_fc_si(bb, attnT_t[bb], ss))(
                                b - 1, si
                            )
                        )
                if b + 1 < NB:
                    for m in (0, 6, 1, 7):
                        fillers.append(
                            (lambda bb, mm: lambda: emit_qkT_mtile(bb, mm))(b + 1, m)
                        )
                    fillers.append(
                        (lambda bb: lambda: v_t.__setitem__(bb, emit_v(bb)))(b + 1)
                    )

                # pace the fillers evenly across the batch's weave sites so the
                # PE never starves late in the batch (keeps HAM at full clock)
                nf = len(fillers)
                site_state = {"site": 0, "fed": 0}

                def try_fill():
                    site_state["site"] += 1
                    want = site_state["site"] * nf // N_SITES
                    while site_state["fed"] < want:
                        fillers[site_state["fed"]]()
                        site_state["fed"] += 1

                v_all = v_t[b]
                attnT_t[b] = attnT_all = attnT_pool.tile(
                    [128, NKT * S], BF16, tag="attnT", name="attnT_all"
                )
                expT = {}
                for p in range(H // 2 + 1):
                    if p < H // 2:
                        emit_scores(p, qkT_t[b], expT, try_fill)
                    if p >= 1:
                        emit_attnv(p - 1, v_all, attnT_all, expT)
                # any leftover fillers run before the next batch
                while site_state["fed"] < nf:
                    fillers[site_state["fed"]]()
                    site_state["fed"] += 1
                # next-next batch x load + transpose (DMA-only, self-scheduling;
                # emitted after fc fillers so out DMAs aren't head-of-line
                # blocked on the sync queue)
                if b + 2 < NB:
                    emit_x_dma(b + 2)
                    emit_xT(b + 2)

            for si in range(5):
                emit_fc_si(NB - 1, attnT_t[NB - 1], si)

    nc.compile()
    return nc


_NC_CACHE = None


def _get_nc():
    global _NC_CACHE
    if _NC_CACHE is None:
        _NC_CACHE = build_nc()
    return _NC_CACHE


def kernel(x, w_qkv, b_qkv, w_fc, b_fc, _collect=None):
    nc = _get_nc()
    x = np.ascontiguousarray(np.asarray(x, dtype=np.float32))
    w_qkv = np.ascontiguousarray(np.asarray(w_qkv, dtype=np.float32))
    b_qkv = np.ascontiguousarray(np.asarray(b_qkv, dtype=np.float32))
    w_fc = np.ascontiguousarray(np.asarray(w_fc, dtype=np.float32))
    b_fc = np.ascontiguousarray(np.asarray(b_fc, dtype=np.float32))
    in_maps = [
        {
            "x": x[i * NB : (i + 1) * NB],
            "w_qkv": w_qkv,
            "b_qkv": b_qkv,
            "w_fc": w_fc,
            "b_fc": b_fc,
        }
        for i in range(NCORES)
    ]
    kwargs = dict(_collect) if _collect else {}
    res = run_bass_kernel_spmd(nc, in_maps, core_ids=list(range(NCORES)), **kwargs)
    out = np.concatenate([res.results[i]["out"] for i in range(NCORES)], axis=0)
    if _collect is not None and isinstance(_collect, dict):
        _collect["result"] = res
    return out.astype(np.float32)


if __name__ == "__main__":
    xs = np.random.randn(B, S, D).astype(np.float32)
    lim = 1.0 / np.sqrt(D)
    rng = np.random.default_rng(0)
    wq = rng.uniform(-lim, lim, (D, 3 * D)).astype(np.float32)
    bq = rng.uniform(-lim, lim, (3 * D,)).astype(np.float32)
    wf = rng.uniform(-lim, lim, (D, D)).astype(np.float32)
    bf = rng.uniform(-lim, lim, (D,)).astype(np.float32)
    o = kernel(xs, wq, bq, wf, bf)
    print("out", o.shape, o.dtype)


# revision 25
# speedup vs baseline: 1.0541x; 1.0541x over previous
"""Trainium2 Bass kernel: 12-head attention block (qkv proj -> softmax attn -> fc).

Reference semantics (B=32, S=577, D=768, H=12, Dh=64):
    qkv = x @ w_qkv + b_qkv
    q, k, v = split(qkv); attn = softmax(q k^T / 8) v
    out = attn @ w_fc + b_fc

Sharding: data-parallel over batch across 8 NeuronCores (4 images per core),
weights replicated, no collectives. Compute in bf16 with fp32 PSUM accumulation.

v3 layout strategy per core (all matmuls contract over the partition dim):
  - x is transposed by the DMA XBAR (dma_start_transpose, bf16) into
    xT_all [128, (si dk sl)] -- zero PE/DVE cost. Row block si=4 loads rows
    449:577 so every block is a full 128 rows. x0 arrives f32 on the sync
    HWDGE queue and is cast bf16 on the (startup-idle) ScalarE; x1..3 are
    cast in flight by the gpsimd SWDGE queue.
  - w_qkv streams in k-blocks [128, 2304] split into two column halves on
    two parallel queues: cols 0:1152 f32 on the scalar HWDGE queue (+
    ScalarE cast), cols 1152:2304 (incl. v) casting-DMA on SWDGE. Full-row
    descriptors keep both streams near peak DMA bandwidth.
  - qkT [1536, 577] = w_qkv[:, :1536]^T . xT; evac PSUM->bf16 with
    per-partition bias on DVE (tensor_scalar_add).
  - v [577, 768] natural + per-head ones column so attention row-sums fall
    out of the attn@v matmul for free.
  - scoresT[sk, sq] = kT_h^T . qT_h; heads paired even/odd on disjoint PE
    row groups (concurrent K=64 matmuls). exp on ScalarE (scale folded in).
  - attn_outT[65, sq] = (v_h|1)^T . expT; row 64 = softmax denominators.
  - normalize: reciprocal_approx_fast (DVE) + partition_broadcast + multiply
    on gpsimd (keeps DVE free for PSUM evacuation).
  - fc: out[s, :] = attn_T_k^T . w_fc_k, + b_fc broadcast.

Scheduling: attention alternates PE-light/ScalarE-heavy (scores+exp) with
PE-heavy (attn@v) phases; independent PE work (late qkT tiles, prev-batch fc,
next-batch qkT/v) is woven as filler between scores sk-groups, PACED EVENLY
across the 30 weave sites per batch so the PE stream stays dense end-to-end
(HAM clock-gate stays at full rate). PSUM: two 2-slot pools, 8 banks total.
"""

import os
import sys

import numpy as np

for _p in ("/opt/trn_rl_repo", "/root/.axon_site/_ro/trn_rl_repo"):
    if os.path.isdir(_p) and _p not in sys.path:
        sys.path.insert(0, _p)

import concourse.bass as bass  # noqa: E402
import concourse.tile as tile  # noqa: E402
from concourse import bacc, mybir  # noqa: E402
from concourse.bass_utils import run_bass_kernel_spmd  # noqa: E402
from concourse.masks import make_identity  # noqa: E402

F32 = mybir.dt.float32
BF16 = mybir.dt.bfloat16

B, S, D = 32, 577, 768
H, DH = 12, 64
NCORES = 8
NB = B // NCORES  # 4 batch images per core
SCALE = DH**-0.5  # 0.125
NKT = D // 128  # 6 contraction tiles of 128
S_TILES = [(0, 128), (128, 128), (256, 128), (384, 128), (512, 65)]
# x row blocks for DMA/transpose: all full 128 rows (block 4 overlaps block 3)
X_TILES = [0, 128, 256, 384, 449]
SL4 = 512 - 449  # si=4: column offset of s=512 within its transposed block
CH_S = [(0, 512), (512, 65)]  # 577 split at PSUM-bank boundary
CH_D = [(0, 512), (512, 256)]  # 768 split at PSUM-bank boundary
WSPLIT = 1152  # w_qkv column split between the two startup DMA streams
EXP = mybir.ActivationFunctionType.Exp
COPY = mybir.ActivationFunctionType.Copy


def build_nc():
    nc = bacc.Bacc(None)
    x_ext = nc.declare_dram_parameter("x", [NB, S, D], F32, isOutput=False)
    wqkv_ext = nc.declare_dram_parameter("w_qkv", [D, 3 * D], F32, isOutput=False)
    bqkv_ext = nc.declare_dram_parameter("b_qkv", [3 * D], F32, isOutput=False)
    wfc_ext = nc.declare_dram_parameter("w_fc", [D, D], F32, isOutput=False)
    bfc_ext = nc.declare_dram_parameter("b_fc", [D], F32, isOutput=False)
    out_ext = nc.declare_dram_parameter("out", [NB, S, D], F32, isOutput=True)

    with tile.TileContext(nc) as tc:
        with (
            tc.tile_pool(name="const", bufs=1) as cpool,
            tc.tile_pool(name="x", bufs=2) as x_pool,
            tc.tile_pool(name="xstg", bufs=2) as xstg_pool,
            tc.tile_pool(name="wstg", bufs=2) as wstg_pool,
            tc.tile_pool(name="xT", bufs=2) as xT_pool,
            tc.tile_pool(name="qkT", bufs=2) as qkT_pool,
            tc.tile_pool(name="v", bufs=2) as v_pool,
            tc.tile_pool(name="expT", bufs=5) as expT_pool,
            tc.tile_pool(name="attnT", bufs=2) as attnT_pool,
            tc.tile_pool(name="small", bufs=3) as small_pool,
            tc.tile_pool(name="osb", bufs=2) as osb_pool,
            tc.tile_pool(name="psS", bufs=2, space="PSUM") as psS,
            tc.tile_pool(name="psW", bufs=2, space="PSUM") as psW,
        ):
            x_t = {}
            # w_qkv bf16 k-blocks [128, 2304] (q | k | v columns)
            w_qkv_k = [
                cpool.tile([128, 3 * D], BF16, name=f"wqkv{k}") for k in range(NKT)
            ]
            wfc_k = [cpool.tile([128, D], BF16, name=f"wfc{k}") for k in range(NKT)]

            # ---- startup streams ----
            # constants FIRST: the identity build must head the gpsimd queue
            # (behind the SWDGE descriptor generation it would stall the
            # b_qk matmul at the head of the in-order PE queue for ~25us)
            identity = cpool.tile([12, 12], F32)
            make_identity(nc, identity[:])
            ones = cpool.tile([1, 128], F32)
            nc.vector.memset(ones[:], 1.0)
            # tiny bias DMAs first on sync so they aren't stuck behind staging
            b12 = cpool.tile([12, 128], F32)
            nc.sync.dma_start(
                b12[:], bqkv_ext[0 : 2 * D].rearrange("(m p) -> m p", m=12)
            )
            brow_v = cpool.tile([1, D], F32)
            nc.sync.dma_start(brow_v[:], bqkv_ext[None, 2 * D : 3 * D])
            brow_fc = cpool.tile([1, D], F32)
            nc.sync.dma_start(brow_fc[:], bfc_ext[None, :])

            def emit_x_dma(b):  # SWDGE casting x load
                x_t[b] = x_pool.tile([128, 5 * D], BF16, tag="x", name="x_all")
                for si, xs0 in enumerate(X_TILES):
                    nc.gpsimd.dma_start(
                        x_t[b][:, si * D : (si + 1) * D],
                        x_ext[b, xs0 : xs0 + 128, :],
                    )

            # x0: f32 via the scalar HWDGE queue, cast on ScalarE. Safe from
            # priority inversion: every exp transitively depends on all five
            # x0 casts, so the scheduler cannot order exps ahead of them.
            x_t[0] = x_pool.tile([128, 5 * D], BF16, tag="x", name="x_all")
            xstg = {}
            for si, xs0 in enumerate(X_TILES):
                xstg[si] = xstg_pool.tile([128, D], F32, tag="xstg", name="xstg")
                nc.scalar.dma_start(xstg[si][:], x_ext[0, xs0 : xs0 + 128, :])
            for si in range(5):
                nc.scalar.activation(
                    x_t[0][:, si * D : (si + 1) * D], xstg[si][:], COPY
                )
            # SWDGE: v columns first (attn@v p0 reads v ~27us), then m9..11
            # k-head columns (scores pair 3+, ~45us)
            for k in range(NKT):
                nc.gpsimd.dma_start(
                    w_qkv_k[k][:, 2 * D : 3 * D],
                    wqkv_ext[k * 128 : (k + 1) * 128, 2 * D : 3 * D],
                )
            for k in range(NKT):
                nc.gpsimd.dma_start(
                    w_qkv_k[k][:, WSPLIT : 2 * D],
                    wqkv_ext[k * 128 : (k + 1) * 128, WSPLIT : 2 * D],
                )
            # w_qkv cols 0:1152 f32 staged on the sync HWDGE queue, cast on
            # DVE. The w casts are the ONLY early DVE work, so the staging
            # slot chain schedules cleanly; all steady-state DVE ops
            # transitively need the weights so no inversion is possible.
            wstg = {}
            for k in range(NKT):
                wstg[k] = wstg_pool.tile([128, WSPLIT], F32, tag="wstg", name="wstg")
                nc.sync.dma_start(
                    wstg[k][:], wqkv_ext[k * 128 : (k + 1) * 128, 0:WSPLIT]
                )
            for k in range(NKT):
                nc.vector.tensor_copy(w_qkv_k[k][:, 0:WSPLIT], wstg[k][:])

            # b_qk: PE-transpose the [12,128] bias block to [128,12]
            b_qk = cpool.tile([128, H], F32)
            pbk = psW.tile([128, H], F32, tag="psW", name="pbk")
            nc.tensor.matmul(
                pbk[:, :], lhsT=b12[:, :], rhs=identity[:, :], start=True, stop=True
            )
            nc.vector.tensor_copy(b_qk[:], pbk[:])

            # broadcast biases to all 128 partitions via K=1 matmul
            b_v_bc = cpool.tile([128, D], F32)
            b_fc_bc = cpool.tile([128, D], F32)
            for row, bc in ((brow_v, b_v_bc), (brow_fc, b_fc_bc)):
                pb = psW.tile([128, D], F32, tag="psW", name="pb")
                for c0, cl in CH_D:
                    nc.tensor.matmul(
                        pb[:, c0 : c0 + cl],
                        lhsT=ones[0:1, 0:128],
                        rhs=row[0:1, c0 : c0 + cl],
                        start=True,
                        stop=True,
                    )
                nc.vector.tensor_copy(bc[:], pb[:])

            # ---- xT via DMA XBAR transpose (sync queue) ----
            # xT_all[b][p, si*768 + dk*128 + sl] = x[X_TILES[si]+sl, dk*128+p]
            xT_t = {}

            def emit_xT(b, eng=None):
                # b=0 issues from the scalar queue (nothing queued behind it
                # there); later batches from sync where x_t is already
                # resident by emission time so no head-of-line blocking
                eng = eng or nc.sync
                xT_t[b] = xT_pool.tile([128, 5 * NKT * 128], BF16, tag="xT", name="xT")
                for si in range(5):
                    eng.dma_start_transpose(
                        xT_t[b][:, si * D : (si + 1) * D].rearrange(
                            "p (dk sl) -> p dk sl", dk=NKT
                        ),
                        x_t[b][:, si * D : (si + 1) * D],
                    )

            def xT4(b):
                return xT_t[b][:].rearrange("p (si dk sl) -> p si dk sl", si=5, dk=NKT)

            # rhs APs for qkT: s-chunk (0,512) spans si 0..3; (512,65) is si4
            def xT_schunk(b, k, c0):
                if c0 == 0:
                    return xT4(b)[:, 0:4, k, :]
                return xT4(b)[:, 4, k, SL4 : SL4 + 65]

            # ---- per-batch emission helpers ----
            qkT_t = {}

            def emit_qkT_mtile(b, m, ps=None):
                if b not in qkT_t:
                    qkT_t[b] = {}
                qkT_t[b][m] = qkT_pool.tile(
                    [128, S], BF16, tag=f"qkT{m}", name=f"qkT{m}"
                )
                pqk = (ps or psW).tile([128, S], F32, tag="psW" if ps is None else "psS", name="pqk")
                for k in range(NKT):
                    for c0, cl in CH_S:
                        nc.tensor.matmul(
                            pqk[:, c0 : c0 + cl],
                            lhsT=w_qkv_k[k][:, m * 128 : (m + 1) * 128],
                            rhs=xT_schunk(b, k, c0),
                            start=(k == 0),
                            stop=(k == NKT - 1),
                        )
                nc.vector.tensor_scalar_add(qkT_t[b][m][:], pqk[:], b_qk[:, m : m + 1])

            def emit_v(b):
                # v natural [577, 768] + per-head ones column (65 floats per head)
                v_all = v_pool.tile(
                    [128, 5 * H * (DH + 1)], BF16, tag="v", name="v_all"
                )
                v4 = v_all[:].rearrange("p (s h e) -> p s h e", s=5, h=H)
                nc.vector.memset(v4[:, :, :, DH : DH + 1], 1.0)
                for si, (s0, psl) in enumerate(S_TILES):
                    pv = psW.tile([128, D], F32, tag="psW", name="pv")
                    sl0 = SL4 if si == 4 else 0
                    for k in range(NKT):
                        for c0, cl in CH_D:
                            nc.tensor.matmul(
                                pv[0:psl, c0 : c0 + cl],
                                lhsT=xT4(b)[:, si, k, sl0 : sl0 + psl],
                                rhs=w_qkv_k[k][:, 2 * D + c0 : 2 * D + c0 + cl],
                                start=(k == 0),
                                stop=(k == NKT - 1),
                            )
                    nc.vector.tensor_add(
                        v4[0:psl, si, :, 0:DH],
                        pv[0:psl, :].rearrange("p (h e) -> p h e", h=H),
                        b_v_bc[0:psl, :].rearrange("p (h e) -> p h e", h=H),
                    )
                return v_all

            def emit_scores(p, qkT_all, expT, try_fill=None):
                heads = (2 * p, 2 * p + 1)
                for h in heads:
                    expT[h] = expT_pool.tile(
                        [128, 5 * S], BF16, tag="expT", name=f"expT{h % 2}"
                    )
                for si, (s0, psl) in enumerate(S_TILES):
                    psc = {}
                    for h in heads:
                        psc[h] = psS.tile([128, S], F32, tag="psS", name=f"psc{h % 2}")
                    for c0, cl in CH_S:
                        for h in heads:
                            hoff = (h % 2) * 64
                            qm, km = h // 2, NKT + h // 2
                            nc.tensor.matmul(
                                psc[h][0:psl, c0 : c0 + cl],
                                lhsT=qkT_all[km][hoff : hoff + 64, s0 : s0 + psl],
                                rhs=qkT_all[qm][hoff : hoff + 64, c0 : c0 + cl],
                                start=True,
                                stop=True,
                            )
                    for h in heads:
                        nc.scalar.activation(
                            expT[h][0:psl, si * S : (si + 1) * S],
                            psc[h][0:psl, :],
                            EXP,
                            scale=float(SCALE),
                        )
                    if try_fill is not None:
                        try_fill()

            def emit_attnv(p, v_all, attnT_all, expT):
                heads = (2 * p, 2 * p + 1)
                rinv = {}
                for h in heads:
                    hoff = (h % 2) * 64
                    # attn_outT [65, 577]: rows 0:64 = out^T unnorm, row 64 = sums
                    po = psW.tile([65, S], F32, tag="psW", name="po")
                    for si, (s0, psl) in enumerate(S_TILES):
                        for c0, cl in CH_S:
                            nc.tensor.matmul(
                                po[:, c0 : c0 + cl],
                                lhsT=v_all[
                                    0:psl,
                                    si * H * (DH + 1)
                                    + h * (DH + 1) : si * H * (DH + 1)
                                    + (h + 1) * (DH + 1),
                                ],
                                rhs=expT[h][0:psl, si * S + c0 : si * S + c0 + cl],
                                start=(si == 0),
                                stop=(si == 4),
                            )
                    # drain po fast: unnormalized copy + staged fast reciprocal
                    nc.vector.tensor_copy(
                        attnT_all[hoff : hoff + 64, (h // 2) * S : (h // 2 + 1) * S],
                        po[0:64, :],
                    )
                    rs = small_pool.tile([1, S], F32, tag="rs", name=f"rs{h % 2}")
                    nc.vector.tensor_copy(rs[:], po[64:65, :])
                    rinv[h] = small_pool.tile(
                        [1, S], F32, tag="rinv", name=f"rinv{h % 2}"
                    )
                    nc.vector.reciprocal_approx_fast(rinv[h][:], rs[:])
                    del expT[h]
                for h in heads:
                    hoff = (h % 2) * 64
                    rbc = small_pool.tile([128, S], F32, tag="rbc")
                    nc.gpsimd.partition_broadcast(rbc[:, :], rinv[h][0:1, :])
                    nc.vector.tensor_mul(
                        attnT_all[hoff : hoff + 64, (h // 2) * S : (h // 2 + 1) * S],
                        attnT_all[hoff : hoff + 64, (h // 2) * S : (h // 2 + 1) * S],
                        rbc[hoff : hoff + 64, :],
                    )

            def emit_fc_si(b, attnT_all, si):
                s0, psl = S_TILES[si]
                pf = psW.tile([128, D], F32, tag="psW", name="pf")
                for k in range(NKT):
                    for c0, cl in CH_D:
                        nc.tensor.matmul(
                            pf[0:psl, c0 : c0 + cl],
                            lhsT=attnT_all[:, k * S + s0 : k * S + s0 + psl],
                            rhs=wfc_k[k][:, c0 : c0 + cl],
                            start=(k == 0),
                            stop=(k == NKT - 1),
                        )
                osb = osb_pool.tile([128, D], F32, tag="osb")
                nc.vector.tensor_add(osb[0:psl, :], pf[0:psl, :], b_fc_bc[0:psl, :])
                nc.sync.dma_start(out_ext[b, s0 : s0 + psl, :], osb[0:psl, :])

            # ---- schedule ----
            emit_xT(0, nc.scalar)
            # scores pair 0/1 tiles up front; m1/m7 borrow the (still idle)
            # scores PSUM slots so all four accumulations chase the weight
            # stream concurrently; rest woven as fillers
            emit_qkT_mtile(0, 0)
            emit_qkT_mtile(0, 6)
            emit_qkT_mtile(0, 1, ps=psS)
            emit_qkT_mtile(0, 7, ps=psS)
            # SWDGE continues: x1, then wfc (needed ~100us in), then x2/x3
            # are emitted inside the batch loop
            emit_x_dma(1)
            emit_xT(1)
            for k in range(NKT):
                nc.gpsimd.dma_start(wfc_k[k][:], wfc_ext[k * 128 : (k + 1) * 128, :])
            v_t = {}
            attnT_t = {}
            v_t[0] = emit_v(0)

            N_SITES = (H // 2) * 5  # try_fill call sites per batch

            for b in range(NB):
                fillers = []
                # this batch's remaining qkT tiles, 2 per step, one step ahead
                # of the scores pair that reads them (pair p needs m=p, 6+p)
                for p in range(2, NKT):
                    fillers.append(
                        (lambda bb, mm: lambda: emit_qkT_mtile(bb, mm))(b, p)
                    )
                    fillers.append(
                        (lambda bb, mm: lambda: emit_qkT_mtile(bb, mm))(b, NKT + p)
                    )
                if b >= 1:
                    for si in range(5):
                        fillers.append(
                            (lambda bb, ss: lambda: emit_fc_si(bb, attnT_t[bb], ss))(
                                b - 1, si
                            )
                        )
                if b + 1 < NB:
                    for m in (0, 6, 1, 7):
                        fillers.append(
                            (lambda bb, mm: lambda: emit_qkT_mtile(bb, mm))(b + 1, m)
                        )
                    fillers.append(
                        (lambda bb: lambda: v_t.__setitem__(bb, emit_v(bb)))(b + 1)
                    )

                # pace the fillers evenly across the batch's weave sites so the
                # PE never starves late in the batch (keeps HAM at full clock)
                nf = len(fillers)
                site_state = {"site": 0, "fed": 0}

                def try_fill():
                    site_state["site"] += 1
                    want = site_state["site"] * nf // N_SITES
                    while site_state["fed"] < want:
                        fillers[site_state["fed"]]()
                        site_state["fed"] += 1

                v_all = v_t[b]
                attnT_t[b] = attnT_all = attnT_pool.tile(
                    [128, NKT * S], BF16, tag="attnT", name="attnT_all"
                )
                expT = {}
                for p in range(H // 2 + 1):
                    if p < H // 2:
                        emit_scores(p, qkT_t[b], expT, try_fill)
                    if p >= 1:
                        emit_attnv(p - 1, v_all, attnT_all, expT)
                # any leftover fillers run before the next batch
                while site_state["fed"] < nf:
                    fillers[site_state["fed"]]()
                    site_state["fed"] += 1
                # next-next batch x load + transpose (DMA-only, self-scheduling;
                # emitted after fc fillers so out DMAs aren't head-of-line
                # blocked on the sync queue)
                if b + 2 < NB:
                    emit_x_dma(b + 2)
                    emit_xT(b + 2)

            for si in range(5):
                emit_fc_si(NB - 1, attnT_t[NB - 1], si)

    nc.compile()
    return nc


_NC_CACHE = None


def _get_nc():
    global _NC_CACHE
    if _NC_CACHE is None:
        _NC_CACHE = build_nc()
    return _NC_CACHE


def kernel(x, w_qkv, b_qkv, w_fc, b_fc, _collect=None):
    nc = _get_nc()
    x = np.ascontiguousarray(np.asarray(x, dtype=np.float32))
    w_qkv = np.ascontiguousarray(np.asarray(w_qkv, dtype=np.float32))
    b_qkv = np.ascontiguousarray(np.asarray(b_qkv, dtype=np.float32))
    w_fc = np.ascontiguousarray(np.asarray(w_fc, dtype=np.float32))
    b_fc = np.ascontiguousarray(np.asarray(b_fc, dtype=np.float32))
    in_maps = [
        {
            "x": x[i * NB : (i + 1) * NB],
            "w_qkv": w_qkv,
            "b_qkv": b_qkv,
            "w_fc": w_fc,
            "b_fc": b_fc,
        }
        for i in range(NCORES)
    ]
    kwargs = dict(_collect) if _collect else {}
    res = run_bass_kernel_spmd(nc, in_maps, core_ids=list(range(NCORES)), **kwargs)
    out = np.concatenate([res.results[i]["out"] for i in range(NCORES)], axis=0)
    if _collect is not None and isinstance(_collect, dict):
        _collect["result"] = res
    return out.astype(np.float32)


if __name__ == "__main__":
    xs = np.random.randn(B, S, D).astype(np.float32)
    lim = 1.0 / np.sqrt(D)
    rng = np.random.default_rng(0)
    wq = rng.uniform(-lim, lim, (D, 3 * D)).astype(np.float32)
    bq = rng.uniform(-lim, lim, (3 * D,)).astype(np.float32)
    wf = rng.uniform(-lim, lim, (D, D)).astype(np.float32)
    bf = rng.uniform(-lim, lim, (D,)).astype(np.float32)
    o = kernel(xs, wq, bq, wf, bf)
    print("out", o.shape, o.dtype)


# revision 31
# speedup vs baseline: 1.1068x; 1.0500x over previous
"""Trainium2 Bass kernel: 12-head attention block (qkv proj -> softmax attn -> fc).

Reference semantics (B=32, S=577, D=768, H=12, Dh=64):
    qkv = x @ w_qkv + b_qkv
    q, k, v = split(qkv); attn = softmax(q k^T / 8) v
    out = attn @ w_fc + b_fc

Sharding: data-parallel over batch across 8 NeuronCores (4 images per core),
weights replicated, no collectives. Compute in bf16 with fp32 PSUM accumulation.

v3 layout strategy per core (all matmuls contract over the partition dim):
  - x is transposed by the DMA XBAR (dma_start_transpose, bf16) into
    xT_all [128, (si dk sl)] -- zero PE/DVE cost. Row block si=4 loads rows
    449:577 so every block is a full 128 rows. x0 arrives f32 on the sync
    HWDGE queue and is cast bf16 on the (startup-idle) ScalarE; x1..3 are
    cast in flight by the gpsimd SWDGE queue.
  - w_qkv streams in k-blocks [128, 2304] split into two column halves on
    two parallel queues: cols 0:1152 f32 on the scalar HWDGE queue (+
    ScalarE cast), cols 1152:2304 (incl. v) casting-DMA on SWDGE. Full-row
    descriptors keep both streams near peak DMA bandwidth.
  - qkT [1536, 577] = w_qkv[:, :1536]^T . xT; evac PSUM->bf16 with
    per-partition bias on DVE (tensor_scalar_add).
  - v [577, 768] natural + per-head ones column so attention row-sums fall
    out of the attn@v matmul for free.
  - scoresT[sk, sq] = kT_h^T . qT_h; heads paired even/odd on disjoint PE
    row groups (concurrent K=64 matmuls). exp on ScalarE (scale folded in).
  - attn_outT[65, sq] = (v_h|1)^T . expT; row 64 = softmax denominators.
  - normalize: reciprocal_approx_fast (DVE) + partition_broadcast + multiply
    on gpsimd (keeps DVE free for PSUM evacuation).
  - fc: out[s, :] = attn_T_k^T . w_fc_k, + b_fc broadcast.

Scheduling: attention alternates PE-light/ScalarE-heavy (scores+exp) with
PE-heavy (attn@v) phases; independent PE work (late qkT tiles, prev-batch fc,
next-batch qkT/v) is woven as filler between scores sk-groups, PACED EVENLY
across the 30 weave sites per batch so the PE stream stays dense end-to-end
(HAM clock-gate stays at full rate). PSUM: two 2-slot pools, 8 banks total.
"""

import os
import sys

import numpy as np

for _p in ("/opt/trn_rl_repo", "/root/.axon_site/_ro/trn_rl_repo"):
    if os.path.isdir(_p) and _p not in sys.path:
        sys.path.insert(0, _p)

import concourse.bass as bass  # noqa: E402
import concourse.tile as tile  # noqa: E402
from concourse import bacc, mybir  # noqa: E402
from concourse.bass_utils import run_bass_kernel_spmd  # noqa: E402
from concourse.masks import make_identity  # noqa: E402

F32 = mybir.dt.float32
BF16 = mybir.dt.bfloat16

B, S, D = 32, 577, 768
H, DH = 12, 64
NCORES = 8
NB = B // NCORES  # 4 batch images per core
SCALE = DH**-0.5  # 0.125
NKT = D // 128  # 6 contraction tiles of 128
S_TILES = [(0, 128), (128, 128), (256, 128), (384, 128), (512, 65)]
# x row blocks for DMA/transpose: all full 128 rows (block 4 overlaps block 3)
X_TILES = [0, 128, 256, 384, 449]
SL4 = 512 - 449  # si=4: column offset of s=512 within its transposed block
CH_S = [(0, 512), (512, 65)]  # 577 split at PSUM-bank boundary
CH_D = [(0, 512), (512, 256)]  # 768 split at PSUM-bank boundary
WSPLIT = 1152  # w_qkv column split between the two startup DMA streams
EXP = mybir.ActivationFunctionType.Exp
COPY = mybir.ActivationFunctionType.Copy


def build_nc():
    nc = bacc.Bacc(None)
    x_ext = nc.declare_dram_parameter("x", [NB, S, D], F32, isOutput=False)
    wqkv_ext = nc.declare_dram_parameter("w_qkv", [D, 3 * D], F32, isOutput=False)
    bqkv_ext = nc.declare_dram_parameter("b_qkv", [3 * D], F32, isOutput=False)
    wfc_ext = nc.declare_dram_parameter("w_fc", [D, D], F32, isOutput=False)
    bfc_ext = nc.declare_dram_parameter("b_fc", [D], F32, isOutput=False)
    out_ext = nc.declare_dram_parameter("out", [NB, S, D], F32, isOutput=True)

    with tile.TileContext(nc) as tc:
        with (
            tc.tile_pool(name="const", bufs=1) as cpool,
            tc.tile_pool(name="x", bufs=2) as x_pool,
            tc.tile_pool(name="xstg", bufs=5) as xstg_pool,
            tc.tile_pool(name="xT", bufs=2) as xT_pool,
            tc.tile_pool(name="qkT", bufs=2) as qkT_pool,
            tc.tile_pool(name="v", bufs=2) as v_pool,
            tc.tile_pool(name="expT", bufs=5) as expT_pool,
            tc.tile_pool(name="attnT", bufs=2) as attnT_pool,
            tc.tile_pool(name="small", bufs=2) as small_pool,
            tc.tile_pool(name="osb", bufs=2) as osb_pool,
            tc.tile_pool(name="psS", bufs=2, space="PSUM") as psS,
            tc.tile_pool(name="psW", bufs=2, space="PSUM") as psW,
        ):
            x_t = {}
            # w_qkv bf16 k-blocks [128, 2304] (q | k | v columns)
            w_qkv_k = [
                cpool.tile([128, 3 * D], BF16, name=f"wqkv{k}") for k in range(NKT)
            ]
            wfc_k = [cpool.tile([128, D], BF16, name=f"wfc{k}") for k in range(NKT)]

            # ---- startup streams ----
            # constants FIRST: the identity build must head the gpsimd queue
            # (behind the SWDGE descriptor generation it would stall the
            # b_qk matmul at the head of the in-order PE queue for ~25us)
            identity = cpool.tile([128, 128], F32)
            make_identity(nc, identity[:])
            ones = cpool.tile([1, 128], F32)
            nc.vector.memset(ones[:], 1.0)
            # tiny bias DMAs first on sync so they aren't stuck behind staging
            b12 = cpool.tile([12, 128], F32)
            nc.sync.dma_start(
                b12[:], bqkv_ext[0 : 2 * D].rearrange("(m p) -> m p", m=12)
            )
            brow_v = cpool.tile([1, D], F32)
            nc.sync.dma_start(brow_v[:], bqkv_ext[None, 2 * D : 3 * D])
            brow_fc = cpool.tile([1, D], F32)
            nc.sync.dma_start(brow_fc[:], bfc_ext[None, :])

            def emit_x_dma(b):  # SWDGE casting x load (b >= 2)
                x_t[b] = x_pool.tile([128, 5 * D], BF16, tag="x", name="x_all")
                for si, xs0 in enumerate(X_TILES):
                    nc.gpsimd.dma_start(
                        x_t[b][:, si * D : (si + 1) * D],
                        x_ext[b, xs0 : xs0 + 128, :],
                    )

            # x0/x1: f32 on the sync HWDGE queue; transposed ON THE PE with
            # fp32 identity matmuls (startup filler work while weights
            # stream; no engine casts anywhere the scheduler can misorder)
            xstg = {}
            for b in (0, 1):
                for si, xs0 in enumerate(X_TILES):
                    t = xstg_pool.tile([128, D], F32, tag="xstg", name="xstg")
                    nc.sync.dma_start(t[:], x_ext[b, xs0 : xs0 + 128, :])
                    xstg[(b, si)] = t
            # SWDGE priority order: q + k-heads m6..8 (cols 0:1152) gate the
            # first score pairs; v gates attn@v p0 (~30us); m9..11 gate
            # scores pair 3 (~50us)
            for k in range(NKT):
                nc.gpsimd.dma_start(
                    w_qkv_k[k][:, 0:WSPLIT],
                    wqkv_ext[k * 128 : (k + 1) * 128, 0:WSPLIT],
                )
            for k in range(NKT):
                nc.gpsimd.dma_start(
                    w_qkv_k[k][:, 2 * D : 3 * D],
                    wqkv_ext[k * 128 : (k + 1) * 128, 2 * D : 3 * D],
                )
            for k in range(NKT):
                nc.gpsimd.dma_start(
                    w_qkv_k[k][:, WSPLIT : 2 * D],
                    wqkv_ext[k * 128 : (k + 1) * 128, WSPLIT : 2 * D],
                )

            # b_qk: PE-transpose the [12,128] bias block to [128,12]
            b_qk = cpool.tile([128, H], F32)
            pbk = psW.tile([128, H], F32, tag="psW", name="pbk")
            nc.tensor.matmul(
                pbk[:, :], lhsT=b12[:, :], rhs=identity[0:12, 0:12], start=True, stop=True
            )
            nc.vector.tensor_copy(b_qk[:], pbk[:])

            # broadcast biases to all 128 partitions via K=1 matmul
            b_v_bc = cpool.tile([128, D], F32)
            b_fc_bc = cpool.tile([128, D], F32)
            for row, bc in ((brow_v, b_v_bc), (brow_fc, b_fc_bc)):
                pb = psW.tile([128, D], F32, tag="psW", name="pb")
                for c0, cl in CH_D:
                    nc.tensor.matmul(
                        pb[:, c0 : c0 + cl],
                        lhsT=ones[0:1, 0:128],
                        rhs=row[0:1, c0 : c0 + cl],
                        start=True,
                        stop=True,
                    )
                nc.vector.tensor_copy(bc[:], pb[:])

            # ---- xT via DMA XBAR transpose (sync queue) ----
            # xT_all[b][p, si*768 + dk*128 + sl] = x[X_TILES[si]+sl, dk*128+p]
            xT_t = {}

            def emit_xT(b, eng=None):
                # b=0 issues from the scalar queue (nothing queued behind it
                # there); later batches from sync where x_t is already
                # resident by emission time so no head-of-line blocking
                eng = eng or nc.sync
                xT_t[b] = xT_pool.tile([128, 5 * NKT * 128], BF16, tag="xT", name="xT")
                for si in range(5):
                    eng.dma_start_transpose(
                        xT_t[b][:, si * D : (si + 1) * D].rearrange(
                            "p (dk sl) -> p dk sl", dk=NKT
                        ),
                        x_t[b][:, si * D : (si + 1) * D],
                    )

            def emit_pe_xT(b):
                # fp32 identity-matmul transpose for the first two images:
                # px[d, s] accumulates per dk over the 5 row blocks, then two
                # strided DVE casts scatter it into the si-major xT layout
                xT_t[b] = xT_pool.tile([128, 5 * NKT * 128], BF16, tag="xT", name="xT")
                x4 = xT_t[b][:].rearrange("p (si dk sl) -> p si dk sl", si=5, dk=NKT)
                for dk in range(NKT):
                    px = psS.tile([128, S], F32, tag="psS", name="px")
                    for si in range(4):
                        nc.tensor.matmul(
                            px[:, si * 128 : (si + 1) * 128],
                            lhsT=xstg[(b, si)][:, dk * 128 : (dk + 1) * 128],
                            rhs=identity[:, 0:128],
                            start=True,
                            stop=True,
                        )
                    # si=4 covers s 449:577; split at the PSUM bank boundary
                    nc.tensor.matmul(
                        px[:, 449:512],
                        lhsT=xstg[(b, 4)][:, dk * 128 : (dk + 1) * 128],
                        rhs=identity[:, 0:63],
                        start=True,
                        stop=True,
                    )
                    nc.tensor.matmul(
                        px[:, 512:577],
                        lhsT=xstg[(b, 4)][:, dk * 128 : (dk + 1) * 128],
                        rhs=identity[:, 63:128],
                        start=True,
                        stop=True,
                    )
                    nc.vector.tensor_copy(x4[:, 0:4, dk, :], px[:, 0:512])
                    nc.vector.tensor_copy(x4[:, 4, dk, :], px[:, 449:577])

            def xT4(b):
                return xT_t[b][:].rearrange("p (si dk sl) -> p si dk sl", si=5, dk=NKT)

            # rhs APs for qkT: s-chunk (0,512) spans si 0..3; (512,65) is si4
            def xT_schunk(b, k, c0):
                if c0 == 0:
                    return xT4(b)[:, 0:4, k, :]
                return xT4(b)[:, 4, k, SL4 : SL4 + 65]

            # ---- per-batch emission helpers ----
            qkT_t = {}

            def emit_qkT_mtile(b, m, ps=None):
                if b not in qkT_t:
                    qkT_t[b] = {}
                qkT_t[b][m] = qkT_pool.tile(
                    [128, S], BF16, tag=f"qkT{m}", name=f"qkT{m}"
                )
                pqk = (ps or psW).tile([128, S], F32, tag="psW" if ps is None else "psS", name="pqk")
                for k in range(NKT):
                    for c0, cl in CH_S:
                        nc.tensor.matmul(
                            pqk[:, c0 : c0 + cl],
                            lhsT=w_qkv_k[k][:, m * 128 : (m + 1) * 128],
                            rhs=xT_schunk(b, k, c0),
                            start=(k == 0),
                            stop=(k == NKT - 1),
                        )
                nc.vector.tensor_scalar_add(qkT_t[b][m][:], pqk[:], b_qk[:, m : m + 1])

            def emit_v(b):
                # v natural [577, 768] + per-head ones column (65 floats per head)
                v_all = v_pool.tile(
                    [128, 5 * H * (DH + 1)], BF16, tag="v", name="v_all"
                )
                v4 = v_all[:].rearrange("p (s h e) -> p s h e", s=5, h=H)
                nc.vector.memset(v4[:, :, :, DH : DH + 1], 1.0)
                for si, (s0, psl) in enumerate(S_TILES):
                    pv = psW.tile([128, D], F32, tag="psW", name="pv")
                    sl0 = SL4 if si == 4 else 0
                    for k in range(NKT):
                        for c0, cl in CH_D:
                            nc.tensor.matmul(
                                pv[0:psl, c0 : c0 + cl],
                                lhsT=xT4(b)[:, si, k, sl0 : sl0 + psl],
                                rhs=w_qkv_k[k][:, 2 * D + c0 : 2 * D + c0 + cl],
                                start=(k == 0),
                                stop=(k == NKT - 1),
                            )
                    nc.vector.tensor_add(
                        v4[0:psl, si, :, 0:DH],
                        pv[0:psl, :].rearrange("p (h e) -> p h e", h=H),
                        b_v_bc[0:psl, :].rearrange("p (h e) -> p h e", h=H),
                    )
                return v_all

            def emit_scores(p, qkT_all, expT, try_fill=None):
                heads = (2 * p, 2 * p + 1)
                for h in heads:
                    expT[h] = expT_pool.tile(
                        [128, 5 * S], BF16, tag="expT", name=f"expT{h % 2}"
                    )
                for si, (s0, psl) in enumerate(S_TILES):
                    psc = {}
                    for h in heads:
                        psc[h] = psS.tile([128, S], F32, tag="psS", name=f"psc{h % 2}")
                    for c0, cl in CH_S:
                        for h in heads:
                            hoff = (h % 2) * 64
                            qm, km = h // 2, NKT + h // 2
                            nc.tensor.matmul(
                                psc[h][0:psl, c0 : c0 + cl],
                                lhsT=qkT_all[km][hoff : hoff + 64, s0 : s0 + psl],
                                rhs=qkT_all[qm][hoff : hoff + 64, c0 : c0 + cl],
                                start=True,
                                stop=True,
                            )
                    for h in heads:
                        nc.scalar.activation(
                            expT[h][0:psl, si * S : (si + 1) * S],
                            psc[h][0:psl, :],
                            EXP,
                            scale=float(SCALE),
                        )
                    if try_fill is not None:
                        try_fill()

            def emit_attnv(p, v_all, attnT_all, expT):
                heads = (2 * p, 2 * p + 1)
                rinv = {}
                for h in heads:
                    hoff = (h % 2) * 64
                    # attn_outT [65, 577]: rows 0:64 = out^T unnorm, row 64 = sums
                    po = psW.tile([65, S], F32, tag="psW", name="po")
                    for si, (s0, psl) in enumerate(S_TILES):
                        for c0, cl in CH_S:
                            nc.tensor.matmul(
                                po[:, c0 : c0 + cl],
                                lhsT=v_all[
                                    0:psl,
                                    si * H * (DH + 1)
                                    + h * (DH + 1) : si * H * (DH + 1)
                                    + (h + 1) * (DH + 1),
                                ],
                                rhs=expT[h][0:psl, si * S + c0 : si * S + c0 + cl],
                                start=(si == 0),
                                stop=(si == 4),
                            )
                    # drain po fast: unnormalized copy + staged fast reciprocal
                    nc.vector.tensor_copy(
                        attnT_all[hoff : hoff + 64, (h // 2) * S : (h // 2 + 1) * S],
                        po[0:64, :],
                    )
                    rs = small_pool.tile([1, S], F32, tag="rs", name=f"rs{h % 2}")
                    nc.vector.tensor_copy(rs[:], po[64:65, :])
                    rinv[h] = small_pool.tile(
                        [1, S], F32, tag="rinv", name=f"rinv{h % 2}"
                    )
                    nc.vector.reciprocal_approx_fast(rinv[h][:], rs[:])
                    del expT[h]
                for h in heads:
                    hoff = (h % 2) * 64
                    rbc = small_pool.tile([128, S], F32, tag="rbc")
                    nc.gpsimd.partition_broadcast(rbc[:, :], rinv[h][0:1, :])
                    nc.vector.tensor_mul(
                        attnT_all[hoff : hoff + 64, (h // 2) * S : (h // 2 + 1) * S],
                        attnT_all[hoff : hoff + 64, (h // 2) * S : (h // 2 + 1) * S],
                        rbc[hoff : hoff + 64, :],
                    )

            def emit_fc_si(b, attnT_all, si):
                s0, psl = S_TILES[si]
                pf = psW.tile([128, D], F32, tag="psW", name="pf")
                for k in range(NKT):
                    for c0, cl in CH_D:
                        nc.tensor.matmul(
                            pf[0:psl, c0 : c0 + cl],
                            lhsT=attnT_all[:, k * S + s0 : k * S + s0 + psl],
                            rhs=wfc_k[k][:, c0 : c0 + cl],
                            start=(k == 0),
                            stop=(k == NKT - 1),
                        )
                osb = osb_pool.tile([128, D], F32, tag="osb")
                nc.vector.tensor_add(osb[0:psl, :], pf[0:psl, :], b_fc_bc[0:psl, :])
                nc.sync.dma_start(out_ext[b, s0 : s0 + psl, :], osb[0:psl, :])

            # ---- schedule ----
            # x0/x1 PE transposes are the startup filler while weights
            # stream; the first qkT accumulations chase the weight k-blocks
            # (m1/m7 borrow the still-idle scores PSUM slots)
            emit_pe_xT(0)
            emit_qkT_mtile(0, 0)
            emit_qkT_mtile(0, 6)
            emit_pe_xT(1)
            emit_qkT_mtile(0, 1, ps=psS)
            emit_qkT_mtile(0, 7, ps=psS)
            emit_qkT_mtile(0, 2)
            emit_qkT_mtile(0, 8)
            # SWDGE continues: wfc (needed ~100us), then x2/x3 in the loop
            for k in range(NKT):
                nc.gpsimd.dma_start(wfc_k[k][:], wfc_ext[k * 128 : (k + 1) * 128, :])
            v_t = {}
            attnT_t = {}
            v_t[0] = emit_v(0)

            N_SITES = (H // 2) * 5  # try_fill call sites per batch

            for b in range(NB):
                fillers = []
                # this batch's remaining qkT tiles, 2 per step, one step ahead
                # of the scores pair that reads them (pair p needs m=p, 6+p);
                # batch 0's pair-2 tiles were already emitted at startup
                for p in range(3 if b == 0 else 2, NKT):
                    fillers.append(
                        (lambda bb, mm: lambda: emit_qkT_mtile(bb, mm))(b, p)
                    )
                    fillers.append(
                        (lambda bb, mm: lambda: emit_qkT_mtile(bb, mm))(b, NKT + p)
                    )
                if b >= 1:
                    for si in range(5):
                        fillers.append(
                            (lambda bb, ss: lambda: emit_fc_si(bb, attnT_t[bb], ss))(
                                b - 1, si
                            )
                        )
                if b + 1 < NB:
                    for m in (0, 6, 1, 7):
                        fillers.append(
                            (lambda bb, mm: lambda: emit_qkT_mtile(bb, mm))(b + 1, m)
                        )
                    fillers.append(
                        (lambda bb: lambda: v_t.__setitem__(bb, emit_v(bb)))(b + 1)
                    )

                # pace the fillers evenly across the batch's weave sites so the
                # PE never starves late in the batch (keeps HAM at full clock)
                nf = len(fillers)
                site_state = {"site": 0, "fed": 0}

                def try_fill():
                    site_state["site"] += 1
                    want = site_state["site"] * nf // N_SITES
                    while site_state["fed"] < want:
                        fillers[site_state["fed"]]()
                        site_state["fed"] += 1

                v_all = v_t[b]
                attnT_t[b] = attnT_all = attnT_pool.tile(
                    [128, NKT * S], BF16, tag="attnT", name="attnT_all"
                )
                expT = {}
                for p in range(H // 2 + 1):
                    if p < H // 2:
                        emit_scores(p, qkT_t[b], expT, try_fill)
                    if p >= 1:
                        emit_attnv(p - 1, v_all, attnT_all, expT)
                # any leftover fillers run before the next batch
                while site_state["fed"] < nf:
                    fillers[site_state["fed"]]()
                    site_state["fed"] += 1
                # next-next batch x load + transpose (DMA-only, self-scheduling;
                # emitted after fc fillers so out DMAs aren't head-of-line
                # blocked on the sync queue)
                if b + 2 < NB:
                    emit_x_dma(b + 2)
                    emit_xT(b + 2)

            for si in range(5):
                emit_fc_si(NB - 1, attnT_t[NB - 1], si)

    nc.compile()
    return nc


_NC_CACHE = None


def _get_nc():
    global _NC_CACHE
    if _NC_CACHE is None:
        _NC_CACHE = build_nc()
    return _NC_CACHE


def kernel(x, w_qkv, b_qkv, w_fc, b_fc, _collect=None):
    nc = _get_nc()
    x = np.ascontiguousarray(np.asarray(x, dtype=np.float32))
    w_qkv = np.ascontiguousarray(np.asarray(w_qkv, dtype=np.float32))
    b_qkv = np.ascontiguousarray(np.asarray(b_qkv, dtype=np.float32))
    w_fc = np.ascontiguousarray(np.asarray(w_fc, dtype=np.float32))
    b_fc = np.ascontiguousarray(np.asarray(b_fc, dtype=np.float32))
    in_maps = [
        {
            "x": x[i * NB : (i + 1) * NB],
            "w_qkv": w_qkv,
            "b_qkv": b_qkv,
            "w_fc": w_fc,
            "b_fc": b_fc,
        }
        for i in range(NCORES)
    ]
    kwargs = dict(_collect) if _collect else {}
    res = run_bass_kernel_spmd(nc, in_maps, core_ids=list(range(NCORES)), **kwargs)
    out = np.concatenate([res.results[i]["out"] for i in range(NCORES)], axis=0)
    if _collect is not None and isinstance(_collect, dict):
        _collect["result"] = res
    return out.astype(np.float32)


if __name__ == "__main__":
    xs = np.random.randn(B, S, D).astype(np.float32)
    lim = 1.0 / np.sqrt(D)
    rng = np.random.default_rng(0)
    wq = rng.uniform(-lim, lim, (D, 3 * D)).astype(np.float32)
    bq = rng.uniform(-lim, lim, (3 * D,)).astype(np.float32)
    wf = rng.uniform(-lim, lim, (D, D)).astype(np.float32)
    bf = rng.uniform(-lim, lim, (D,)).astype(np.float32)
    o = kernel(xs, wq, bq, wf, bf)
    print("out", o.shape, o.dtype)


# revision 33
# speedup vs baseline: 1.1295x; 1.0205x over previous
"""Trainium2 Bass kernel: 12-head attention block (qkv proj -> softmax attn -> fc).

Reference semantics (B=32, S=577, D=768, H=12, Dh=64):
    qkv = x @ w_qkv + b_qkv
    q, k, v = split(qkv); attn = softmax(q k^T / 8) v
    out = attn @ w_fc + b_fc

Sharding: data-parallel over batch across 8 NeuronCores (4 images per core),
weights replicated, no collectives. Compute in bf16 with fp32 PSUM accumulation.

v3 layout strategy per core (all matmuls contract over the partition dim):
  - x is transposed by the DMA XBAR (dma_start_transpose, bf16) into
    xT_all [128, (si dk sl)] -- zero PE/DVE cost. Row block si=4 loads rows
    449:577 so every block is a full 128 rows. x0 arrives f32 on the sync
    HWDGE queue and is cast bf16 on the (startup-idle) ScalarE; x1..3 are
    cast in flight by the gpsimd SWDGE queue.
  - w_qkv streams in k-blocks [128, 2304] split into two column halves on
    two parallel queues: cols 0:1152 f32 on the scalar HWDGE queue (+
    ScalarE cast), cols 1152:2304 (incl. v) casting-DMA on SWDGE. Full-row
    descriptors keep both streams near peak DMA bandwidth.
  - qkT [1536, 577] = w_qkv[:, :1536]^T . xT; evac PSUM->bf16 with
    per-partition bias on DVE (tensor_scalar_add).
  - v [577, 768] natural + per-head ones column so attention row-sums fall
    out of the attn@v matmul for free.
  - scoresT[sk, sq] = kT_h^T . qT_h; heads paired even/odd on disjoint PE
    row groups (concurrent K=64 matmuls). exp on ScalarE (scale folded in).
  - attn_outT[65, sq] = (v_h|1)^T . expT; row 64 = softmax denominators.
  - normalize: reciprocal_approx_fast (DVE) + partition_broadcast + multiply
    on gpsimd (keeps DVE free for PSUM evacuation).
  - fc: out[s, :] = attn_T_k^T . w_fc_k, + b_fc broadcast.

Scheduling: attention alternates PE-light/ScalarE-heavy (scores+exp) with
PE-heavy (attn@v) phases; independent PE work (late qkT tiles, prev-batch fc,
next-batch qkT/v) is woven as filler between scores sk-groups, PACED EVENLY
across the 30 weave sites per batch so the PE stream stays dense end-to-end
(HAM clock-gate stays at full rate). PSUM: two 2-slot pools, 8 banks total.
"""

import os
import sys

import numpy as np

for _p in ("/opt/trn_rl_repo", "/root/.axon_site/_ro/trn_rl_repo"):
    if os.path.isdir(_p) and _p not in sys.path:
        sys.path.insert(0, _p)

import concourse.bass as bass  # noqa: E402
import concourse.tile as tile  # noqa: E402
from concourse import bacc, mybir  # noqa: E402
from concourse.bass_utils import run_bass_kernel_spmd  # noqa: E402
from concourse.masks import make_identity  # noqa: E402

F32 = mybir.dt.float32
BF16 = mybir.dt.bfloat16

B, S, D = 32, 577, 768
H, DH = 12, 64
NCORES = 8
NB = B // NCORES  # 4 batch images per core
SCALE = DH**-0.5  # 0.125
NKT = D // 128  # 6 contraction tiles of 128
S_TILES = [(0, 128), (128, 128), (256, 128), (384, 128), (512, 65)]
# x row blocks for DMA/transpose: all full 128 rows (block 4 overlaps block 3)
X_TILES = [0, 128, 256, 384, 449]
SL4 = 512 - 449  # si=4: column offset of s=512 within its transposed block
CH_S = [(0, 512), (512, 65)]  # 577 split at PSUM-bank boundary
CH_D = [(0, 512), (512, 256)]  # 768 split at PSUM-bank boundary
WSPLIT = 1152  # w_qkv column split between the two startup DMA streams
EXP = mybir.ActivationFunctionType.Exp
COPY = mybir.ActivationFunctionType.Copy


def build_nc():
    nc = bacc.Bacc(None)
    x_ext = nc.declare_dram_parameter("x", [NB, S, D], F32, isOutput=False)
    wqkv_ext = nc.declare_dram_parameter("w_qkv", [D, 3 * D], F32, isOutput=False)
    bqkv_ext = nc.declare_dram_parameter("b_qkv", [3 * D], F32, isOutput=False)
    wfc_ext = nc.declare_dram_parameter("w_fc", [D, D], F32, isOutput=False)
    bfc_ext = nc.declare_dram_parameter("b_fc", [D], F32, isOutput=False)
    out_ext = nc.declare_dram_parameter("out", [NB, S, D], F32, isOutput=True)

    with tile.TileContext(nc) as tc:
        with (
            tc.tile_pool(name="const", bufs=1) as cpool,
            tc.tile_pool(name="x", bufs=2) as x_pool,
            tc.tile_pool(name="xstg", bufs=5) as xstg_pool,
            tc.tile_pool(name="xT", bufs=2) as xT_pool,
            tc.tile_pool(name="qkT", bufs=2) as qkT_pool,
            tc.tile_pool(name="v", bufs=2) as v_pool,
            tc.tile_pool(name="expT", bufs=5) as expT_pool,
            tc.tile_pool(name="attnT", bufs=2) as attnT_pool,
            tc.tile_pool(name="small", bufs=2) as small_pool,
            tc.tile_pool(name="osb", bufs=2) as osb_pool,
            tc.tile_pool(name="psS", bufs=2, space="PSUM") as psS,
            tc.tile_pool(name="psW", bufs=2, space="PSUM") as psW,
        ):
            x_t = {}
            # w_qkv bf16 k-blocks [128, 2304] (q | k | v columns)
            w_qkv_k = [
                cpool.tile([128, 3 * D], BF16, name=f"wqkv{k}") for k in range(NKT)
            ]
            wfc_k = [cpool.tile([128, D], BF16, name=f"wfc{k}") for k in range(NKT)]

            # ---- startup streams ----
            # constants FIRST: the identity build must head the gpsimd queue
            # (behind the SWDGE descriptor generation it would stall the
            # b_qk matmul at the head of the in-order PE queue for ~25us)
            identity = cpool.tile([128, 128], F32)
            make_identity(nc, identity[:])
            ones = cpool.tile([1, 128], F32)
            nc.vector.memset(ones[:], 1.0)
            # tiny bias DMAs first on sync so they aren't stuck behind staging
            b12 = cpool.tile([12, 128], F32)
            nc.sync.dma_start(
                b12[:], bqkv_ext[0 : 2 * D].rearrange("(m p) -> m p", m=12)
            )
            brow_v = cpool.tile([1, D], F32)
            nc.sync.dma_start(brow_v[:], bqkv_ext[None, 2 * D : 3 * D])
            brow_fc = cpool.tile([1, D], F32)
            nc.sync.dma_start(brow_fc[:], bfc_ext[None, :])

            def emit_x_dma(b):  # SWDGE casting x load (b >= 2)
                x_t[b] = x_pool.tile([128, 5 * D], BF16, tag="x", name="x_all")
                for si, xs0 in enumerate(X_TILES):
                    nc.gpsimd.dma_start(
                        x_t[b][:, si * D : (si + 1) * D],
                        x_ext[b, xs0 : xs0 + 128, :],
                    )

            # x0/x1: f32 on the sync HWDGE queue; transposed ON THE PE with
            # fp32 identity matmuls (startup filler work while weights
            # stream; no engine casts anywhere the scheduler can misorder)
            xstg = {}
            for b in (0, 1):
                for si, xs0 in enumerate(X_TILES):
                    t = xstg_pool.tile([128, D], F32, tag="xstg", name="xstg")
                    nc.sync.dma_start(t[:], x_ext[b, xs0 : xs0 + 128, :])
                    xstg[(b, si)] = t
            # SWDGE priority order: q + k-heads m6..8 (cols 0:1152) gate the
            # first score pairs; v gates attn@v p0 (~30us); m9..11 gate
            # scores pair 3 (~50us)
            for k in range(NKT):
                nc.gpsimd.dma_start(
                    w_qkv_k[k][:, 0:WSPLIT],
                    wqkv_ext[k * 128 : (k + 1) * 128, 0:WSPLIT],
                )
            for k in range(NKT):
                nc.gpsimd.dma_start(
                    w_qkv_k[k][:, 2 * D : 3 * D],
                    wqkv_ext[k * 128 : (k + 1) * 128, 2 * D : 3 * D],
                )
            for k in range(NKT):
                nc.gpsimd.dma_start(
                    w_qkv_k[k][:, WSPLIT : 2 * D],
                    wqkv_ext[k * 128 : (k + 1) * 128, WSPLIT : 2 * D],
                )

            # b_qk: PE-transpose the [12,128] bias block to [128,12]
            b_qk = cpool.tile([128, H], F32)
            pbk = psW.tile([128, H], F32, tag="psW", name="pbk")
            nc.tensor.matmul(
                pbk[:, :], lhsT=b12[:, :], rhs=identity[0:12, 0:12], start=True, stop=True
            )
            nc.vector.tensor_copy(b_qk[:], pbk[:])

            # broadcast biases to all 128 partitions via K=1 matmul
            b_v_bc = cpool.tile([128, D], F32)
            b_fc_bc = cpool.tile([128, D], F32)
            for row, bc in ((brow_v, b_v_bc), (brow_fc, b_fc_bc)):
                pb = psW.tile([128, D], F32, tag="psW", name="pb")
                for c0, cl in CH_D:
                    nc.tensor.matmul(
                        pb[:, c0 : c0 + cl],
                        lhsT=ones[0:1, 0:128],
                        rhs=row[0:1, c0 : c0 + cl],
                        start=True,
                        stop=True,
                    )
                nc.vector.tensor_copy(bc[:], pb[:])

            # ---- xT via DMA XBAR transpose (sync queue) ----
            # xT_all[b][p, si*768 + dk*128 + sl] = x[X_TILES[si]+sl, dk*128+p]
            xT_t = {}

            def emit_xT(b, eng=None):
                # b=0 issues from the scalar queue (nothing queued behind it
                # there); later batches from sync where x_t is already
                # resident by emission time so no head-of-line blocking
                eng = eng or nc.sync
                xT_t[b] = xT_pool.tile([128, 5 * NKT * 128], BF16, tag="xT", name="xT")
                for si in range(5):
                    eng.dma_start_transpose(
                        xT_t[b][:, si * D : (si + 1) * D].rearrange(
                            "p (dk sl) -> p dk sl", dk=NKT
                        ),
                        x_t[b][:, si * D : (si + 1) * D],
                    )

            def emit_pe_xT(b):
                # fp32 identity-matmul transpose for the first two images:
                # px[d, s] accumulates per dk over the 5 row blocks, then two
                # strided DVE casts scatter it into the si-major xT layout
                xT_t[b] = xT_pool.tile([128, 5 * NKT * 128], BF16, tag="xT", name="xT")
                x4 = xT_t[b][:].rearrange("p (si dk sl) -> p si dk sl", si=5, dk=NKT)
                for dk in range(NKT):
                    px = psS.tile([128, S], F32, tag="psS", name="px")
                    for si in range(4):
                        nc.tensor.matmul(
                            px[:, si * 128 : (si + 1) * 128],
                            lhsT=xstg[(b, si)][:, dk * 128 : (dk + 1) * 128],
                            rhs=identity[:, 0:128],
                            start=True,
                            stop=True,
                        )
                    # si=4 covers s 449:577; split at the PSUM bank boundary
                    nc.tensor.matmul(
                        px[:, 449:512],
                        lhsT=xstg[(b, 4)][:, dk * 128 : (dk + 1) * 128],
                        rhs=identity[:, 0:63],
                        start=True,
                        stop=True,
                    )
                    nc.tensor.matmul(
                        px[:, 512:577],
                        lhsT=xstg[(b, 4)][:, dk * 128 : (dk + 1) * 128],
                        rhs=identity[:, 63:128],
                        start=True,
                        stop=True,
                    )
                    nc.vector.tensor_copy(x4[:, 0:4, dk, :], px[:, 0:512])
                    nc.vector.tensor_copy(x4[:, 4, dk, :], px[:, 449:577])

            def xT4(b):
                return xT_t[b][:].rearrange("p (si dk sl) -> p si dk sl", si=5, dk=NKT)

            # rhs APs for qkT: s-chunk (0,512) spans si 0..3; (512,65) is si4
            def xT_schunk(b, k, c0):
                if c0 == 0:
                    return xT4(b)[:, 0:4, k, :]
                return xT4(b)[:, 4, k, SL4 : SL4 + 65]

            # ---- per-batch emission helpers ----
            qkT_t = {}

            def emit_qkT_mtile(b, m, ps=None):
                if b not in qkT_t:
                    qkT_t[b] = {}
                qkT_t[b][m] = qkT_pool.tile(
                    [128, S], BF16, tag=f"qkT{m}", name=f"qkT{m}"
                )
                pqk = (ps or psW).tile([128, S], F32, tag="psW" if ps is None else "psS", name="pqk")
                for k in range(NKT):
                    for c0, cl in CH_S:
                        nc.tensor.matmul(
                            pqk[:, c0 : c0 + cl],
                            lhsT=w_qkv_k[k][:, m * 128 : (m + 1) * 128],
                            rhs=xT_schunk(b, k, c0),
                            start=(k == 0),
                            stop=(k == NKT - 1),
                        )
                nc.vector.tensor_scalar_add(qkT_t[b][m][:], pqk[:], b_qk[:, m : m + 1])

            def emit_v(b):
                # v natural [577, 768] + per-head ones column (65 floats per head)
                v_all = v_pool.tile(
                    [128, 5 * H * (DH + 1)], BF16, tag="v", name="v_all"
                )
                v4 = v_all[:].rearrange("p (s h e) -> p s h e", s=5, h=H)
                nc.vector.memset(v4[:, :, :, DH : DH + 1], 1.0)
                for si, (s0, psl) in enumerate(S_TILES):
                    pv = psW.tile([128, D], F32, tag="psW", name="pv")
                    sl0 = SL4 if si == 4 else 0
                    for k in range(NKT):
                        for c0, cl in CH_D:
                            nc.tensor.matmul(
                                pv[0:psl, c0 : c0 + cl],
                                lhsT=xT4(b)[:, si, k, sl0 : sl0 + psl],
                                rhs=w_qkv_k[k][:, 2 * D + c0 : 2 * D + c0 + cl],
                                start=(k == 0),
                                stop=(k == NKT - 1),
                            )
                    nc.vector.tensor_add(
                        v4[0:psl, si, :, 0:DH],
                        pv[0:psl, :].rearrange("p (h e) -> p h e", h=H),
                        b_v_bc[0:psl, :].rearrange("p (h e) -> p h e", h=H),
                    )
                return v_all

            def emit_scores(p, qkT_all, expT, try_fill=None):
                heads = (2 * p, 2 * p + 1)
                for h in heads:
                    expT[h] = expT_pool.tile(
                        [128, 5 * S], BF16, tag="expT", name=f"expT{h % 2}"
                    )
                for si, (s0, psl) in enumerate(S_TILES):
                    psc = {}
                    for h in heads:
                        psc[h] = psS.tile([128, S], F32, tag="psS", name=f"psc{h % 2}")
                    for c0, cl in CH_S:
                        for h in heads:
                            hoff = (h % 2) * 64
                            qm, km = h // 2, NKT + h // 2
                            nc.tensor.matmul(
                                psc[h][0:psl, c0 : c0 + cl],
                                lhsT=qkT_all[km][hoff : hoff + 64, s0 : s0 + psl],
                                rhs=qkT_all[qm][hoff : hoff + 64, c0 : c0 + cl],
                                start=True,
                                stop=True,
                            )
                    for h in heads:
                        nc.scalar.activation(
                            expT[h][0:psl, si * S : (si + 1) * S],
                            psc[h][0:psl, :],
                            EXP,
                            scale=float(SCALE),
                        )
                    if try_fill is not None:
                        try_fill()

            def emit_attnv(p, v_all, attnT_all, expT):
                heads = (2 * p, 2 * p + 1)
                rinv = {}
                for h in heads:
                    hoff = (h % 2) * 64
                    # attn_outT [65, 577]: rows 0:64 = out^T unnorm, row 64 = sums
                    po = psW.tile([65, S], F32, tag="psW", name="po")
                    for si, (s0, psl) in enumerate(S_TILES):
                        for c0, cl in CH_S:
                            nc.tensor.matmul(
                                po[:, c0 : c0 + cl],
                                lhsT=v_all[
                                    0:psl,
                                    si * H * (DH + 1)
                                    + h * (DH + 1) : si * H * (DH + 1)
                                    + (h + 1) * (DH + 1),
                                ],
                                rhs=expT[h][0:psl, si * S + c0 : si * S + c0 + cl],
                                start=(si == 0),
                                stop=(si == 4),
                            )
                    # drain po fast: unnormalized copy + staged fast reciprocal
                    nc.vector.tensor_copy(
                        attnT_all[hoff : hoff + 64, (h // 2) * S : (h // 2 + 1) * S],
                        po[0:64, :],
                    )
                    rs = small_pool.tile([1, S], F32, tag="rs", name=f"rs{h % 2}")
                    nc.vector.tensor_copy(rs[:], po[64:65, :])
                    rinv[h] = small_pool.tile(
                        [1, S], F32, tag="rinv", name=f"rinv{h % 2}"
                    )
                    nc.vector.reciprocal_approx_fast(rinv[h][:], rs[:])
                    del expT[h]
                for h in heads:
                    hoff = (h % 2) * 64
                    rbc = small_pool.tile([128, S], F32, tag="rbc")
                    nc.gpsimd.partition_broadcast(rbc[:, :], rinv[h][0:1, :])
                    nc.vector.tensor_mul(
                        attnT_all[hoff : hoff + 64, (h // 2) * S : (h // 2 + 1) * S],
                        attnT_all[hoff : hoff + 64, (h // 2) * S : (h // 2 + 1) * S],
                        rbc[hoff : hoff + 64, :],
                    )

            def emit_fc_si(b, attnT_all, si):
                s0, psl = S_TILES[si]
                pf = psW.tile([128, D], F32, tag="psW", name="pf")
                for k in range(NKT):
                    for c0, cl in CH_D:
                        nc.tensor.matmul(
                            pf[0:psl, c0 : c0 + cl],
                            lhsT=attnT_all[:, k * S + s0 : k * S + s0 + psl],
                            rhs=wfc_k[k][:, c0 : c0 + cl],
                            start=(k == 0),
                            stop=(k == NKT - 1),
                        )
                osb = osb_pool.tile([128, D], F32, tag="osb")
                nc.vector.tensor_add(osb[0:psl, :], pf[0:psl, :], b_fc_bc[0:psl, :])
                nc.sync.dma_start(out_ext[b, s0 : s0 + psl, :], osb[0:psl, :])

            # last-image tail: accumulate fc k-blocks 0..4 while pair 5 is
            # still in exp/attn@v, finish with k=5 after the last normalize
            def emit_fc_partA(attnT_all, si, ps):
                s0, psl = S_TILES[si]
                pf = ps.tile(
                    [128, D], F32, tag="psS" if ps is psS else "psW", name="pf"
                )
                for k in range(NKT - 1):
                    for c0, cl in CH_D:
                        nc.tensor.matmul(
                            pf[0:psl, c0 : c0 + cl],
                            lhsT=attnT_all[:, k * S + s0 : k * S + s0 + psl],
                            rhs=wfc_k[k][:, c0 : c0 + cl],
                            start=(k == 0),
                            stop=(k == NKT - 2),
                        )
                return pf

            def emit_fc_partB(b, attnT_all, si, pf):
                s0, psl = S_TILES[si]
                k = NKT - 1
                for c0, cl in CH_D:
                    nc.tensor.matmul(
                        pf[0:psl, c0 : c0 + cl],
                        lhsT=attnT_all[:, k * S + s0 : k * S + s0 + psl],
                        rhs=wfc_k[k][:, c0 : c0 + cl],
                        start=False,
                        stop=True,
                        skip_group_check=True,
                    )
                osb = osb_pool.tile([128, D], F32, tag="osb")
                nc.vector.tensor_add(osb[0:psl, :], pf[0:psl, :], b_fc_bc[0:psl, :])
                nc.sync.dma_start(out_ext[b, s0 : s0 + psl, :], osb[0:psl, :])

            # ---- schedule ----
            # x0/x1 PE transposes are the startup filler while weights
            # stream; the first qkT accumulations chase the weight k-blocks
            # (m1/m7 borrow the still-idle scores PSUM slots)
            emit_pe_xT(0)
            emit_qkT_mtile(0, 0)
            emit_qkT_mtile(0, 6)
            emit_pe_xT(1)
            emit_qkT_mtile(0, 1, ps=psS)
            emit_qkT_mtile(0, 7, ps=psS)
            emit_qkT_mtile(0, 2)
            emit_qkT_mtile(0, 8)
            # SWDGE continues: wfc (needed ~100us), then x2/x3 in the loop
            for k in range(NKT):
                nc.gpsimd.dma_start(wfc_k[k][:], wfc_ext[k * 128 : (k + 1) * 128, :])
            v_t = {}
            attnT_t = {}
            v_t[0] = emit_v(0)

            N_SITES = (H // 2) * 5  # try_fill call sites per batch

            for b in range(NB):
                fillers = []
                # this batch's remaining qkT tiles, 2 per step, one step ahead
                # of the scores pair that reads them (pair p needs m=p, 6+p);
                # batch 0's pair-2 tiles were already emitted at startup
                for p in range(3 if b == 0 else 2, NKT):
                    fillers.append(
                        (lambda bb, mm: lambda: emit_qkT_mtile(bb, mm))(b, p)
                    )
                    fillers.append(
                        (lambda bb, mm: lambda: emit_qkT_mtile(bb, mm))(b, NKT + p)
                    )
                if b >= 1:
                    for si in range(5):
                        fillers.append(
                            (lambda bb, ss: lambda: emit_fc_si(bb, attnT_t[bb], ss))(
                                b - 1, si
                            )
                        )
                if b + 1 < NB:
                    for m in (0, 6, 1, 7):
                        fillers.append(
                            (lambda bb, mm: lambda: emit_qkT_mtile(bb, mm))(b + 1, m)
                        )
                    fillers.append(
                        (lambda bb: lambda: v_t.__setitem__(bb, emit_v(bb)))(b + 1)
                    )

                # pace the fillers evenly across the batch's weave sites so the
                # PE never starves late in the batch (keeps HAM at full clock)
                nf = len(fillers)
                site_state = {"site": 0, "fed": 0}

                def try_fill():
                    site_state["site"] += 1
                    want = site_state["site"] * nf // N_SITES
                    while site_state["fed"] < want:
                        fillers[site_state["fed"]]()
                        site_state["fed"] += 1

                v_all = v_t[b]
                attnT_t[b] = attnT_all = attnT_pool.tile(
                    [128, NKT * S], BF16, tag="attnT", name="attnT_all"
                )
                expT = {}
                pfA = {}
                for p in range(H // 2 + 1):
                    if p < H // 2:
                        emit_scores(p, qkT_t[b], expT, try_fill)
                    if p == H // 2 and b == NB - 1:
                        # overlap fc k0..4 of the last image with pair 5
                        pfA[0] = emit_fc_partA(attnT_all, 0, psS)
                        pfA[1] = emit_fc_partA(attnT_all, 1, psS)
                    if p >= 1:
                        emit_attnv(p - 1, v_all, attnT_all, expT)
                # any leftover fillers run before the next batch
                while site_state["fed"] < nf:
                    fillers[site_state["fed"]]()
                    site_state["fed"] += 1
                # next-next batch x load + transpose (DMA-only, self-scheduling;
                # emitted after fc fillers so out DMAs aren't head-of-line
                # blocked on the sync queue)
                if b + 2 < NB:
                    emit_x_dma(b + 2)
                    emit_xT(b + 2)

            bL = NB - 1
            pfA[2] = emit_fc_partA(attnT_t[bL], 2, psW)
            pfA[3] = emit_fc_partA(attnT_t[bL], 3, psW)
            for si in range(4):
                emit_fc_partB(bL, attnT_t[bL], si, pfA[si])
            emit_fc_si(bL, attnT_t[bL], 4)

    nc.compile()
    return nc


_NC_CACHE = None


def _get_nc():
    global _NC_CACHE
    if _NC_CACHE is None:
        _NC_CACHE = build_nc()
    return _NC_CACHE


def kernel(x, w_qkv, b_qkv, w_fc, b_fc, _collect=None):
    nc = _get_nc()
    x = np.ascontiguousarray(np.asarray(x, dtype=np.float32))
    w_qkv = np.ascontiguousarray(np.asarray(w_qkv, dtype=np.float32))
    b_qkv = np.ascontiguousarray(np.asarray(b_qkv, dtype=np.float32))
    w_fc = np.ascontiguousarray(np.asarray(w_fc, dtype=np.float32))
    b_fc = np.ascontiguousarray(np.asarray(b_fc, dtype=np.float32))
    in_maps = [
        {
            "x": x[i * NB : (i + 1) * NB],
            "w_qkv": w_qkv,
            "b_qkv": b_qkv,
            "w_fc": w_fc,
            "b_fc": b_fc,
        }
        for i in range(NCORES)
    ]
    kwargs = dict(_collect) if _collect else {}
    res = run_bass_kernel_spmd(nc, in_maps, core_ids=list(range(NCORES)), **kwargs)
    out = np.concatenate([res.results[i]["out"] for i in range(NCORES)], axis=0)
    if _collect is not None and isinstance(_collect, dict):
        _collect["result"] = res
    return out.astype(np.float32)


if __name__ == "__main__":
    xs = np.random.randn(B, S, D).astype(np.float32)
    lim = 1.0 / np.sqrt(D)
    rng = np.random.default_rng(0)
    wq = rng.uniform(-lim, lim, (D, 3 * D)).astype(np.float32)
    bq = rng.uniform(-lim, lim, (3 * D,)).astype(np.float32)
    wf = rng.uniform(-lim, lim, (D, D)).astype(np.float32)
    bf = rng.uniform(-lim, lim, (D,)).astype(np.float32)
    o = kernel(xs, wq, bq, wf, bf)
    print("out", o.shape, o.dtype)
